# revision 89
# baseline (speedup 1.0000x reference)
"""DiT block kernel for TRN2, 8 NeuronCores.

Sharding: DP=4 over batch x TP=2 over heads. Core c: batch b=c//2, half hf=c%2.

Layout: feature-major activations [feature(part), token(free)]; all matmuls chain
as outT = W.T @ actT. Softmax without max-subtraction (scores <= ~4, exp biased
by -1); denominators via ones-column in v^T; per-head reciprocal broadcast by PE.

Dtypes: trunk x fp32r; LN stats fp32r; scores q/k bf16; fp8e4 + DoubleRow for
qkv, q2, ctx-proj, k/v-ctx, proj, out, self-attn P@v, and HALF of fc2's
contraction (fc1 and full-fp8 MLP exceed the 2e-2 budget; half-K fc2 measures
0.0172 on HW vs the 0.02 gate). fp16 output partials summed on host (folds the
fc2 reduce).

adaLN: column-split 8-way (interleaved 128-col chunks, all batches per core) +
one tiny AllToAll (out row j = rank j's col-slice for MY batch -> SPMD-safe
layout). Residual AllReduces carry bf16 deltas only, chunked x4; residual adds
split DVE/gpsimd. LN modulate: h = (x*ksc)*R - S with R/S rank-1 broadcasts
built on PE ([1+sc; sh] lhsT rows x [murs; -1]); modulate TTs split
DVE / ACT-copy+gpsimd per token-half. PSUM: ps_mm 2x2 banks (big matmuls),
ps_aux 4x1 bank (po/pb/R/S/pv per 512-token half) - 1-bank aux tiles are what
let per-head softmax normalize pipeline without blocking the next head.
"""
import sys
import numpy as np

sys.path.insert(0, "/opt/trn_rl_repo")

import ml_dtypes
import concourse.bass as bass
import concourse.mybir as mybir
import concourse.tile as tile
from concourse import bacc
from concourse.bass_utils import run_bass_kernel_spmd

FP32 = mybir.dt.float32
FP32R = mybir.dt.float32r
BF16 = mybir.dt.bfloat16
FP8 = mybir.dt.float8e4
FP16 = mybir.dt.float16
AF = mybir.ActivationFunctionType
ALU = mybir.AluOpType
DR = mybir.MatmulPerfMode.DoubleRow

B, N, D, H, TD, TL = 4, 1024, 1024, 16, 768, 77
HD = 64
EPS = 1e-6
HL = 8          # heads per core
DL = 512        # head-features per core
FFL = 2048      # MLP hidden per core
T = 1024
TLP = 80
NCH = D // 128
PAIR_GROUPS = [[0, 1], [2, 3], [4, 5], [6, 7]]
ALL_GROUP = [[0, 1, 2, 3, 4, 5, 6, 7]]
EXPB = -1.0     # exp bias: P = exp(s - 1), cancels in normalization


def _declare(nc):
    d = {}

    def inp(name, shape, dt):
        d[name] = nc.dram_tensor(name, list(shape), dt, kind="ExternalInput").ap()

    inp("xT", (D, T), FP32R)
    inp("cT", (128, 8, 4), BF16)          # c feature-major, all batches
    inp("teT", (128, 3, 2, TLP), FP8)     # text embed, DR-paired
    inp("w_ada", (6, 128, 8, 128), BF16)  # interleaved col-slice per core
    inp("b_all", (128, 62), FP32)
    inp("w_qk", (4, 128, 4, 2, 256), FP8)
    inp("w_v", (2, 128, 4, 2, 256), FP8)
    inp("b_v", (1, DL), FP32)
    inp("w_proj", (2, 128, 2, 2, 512), FP8)
    inp("w_ctx", (4, 128, 3, 2, 256), FP8)
    inp("w_q", (2, 128, 4, 2, 256), FP8)
    inp("w_k", (2, 128, 4, 2, 256), FP8)
    inp("w_vc", (2, 128, 4, 2, 256), FP8)
    inp("w_out", (2, 128, 2, 2, 512), FP8)
    inp("w_fc1", (8, 128, 8, 256), BF16)
    inp("w_fc2a", (4, 128, 4, 2, 256), FP8)
    inp("w_fc2b", (4, 128, 8, 256), BF16)
    inp("ones_r", (128, 2), FP32R)
    inp("ones2", (34, 128), FP32R)         # block-diag per-head-pair broadcast
    inp("ones_8", (128, 8), FP8)
    inp("ones_b", (128, 8), BF16)
    inp("row_const", (2, 1024), FP32R)  # [ones, -ones]
    inp("maskT", (128, 1), FP32)
    d["out_xT"] = nc.dram_tensor("out_xT", [D, T], FP16, kind="ExternalOutput").ap()
    return d


def _emit(tc, io, pools, nocc=False):
    nc = tc.nc
    sb = pools["sb"]
    xp, hp, h3p, qkp, vtp, atp, pp = (pools[k] for k in
                                      ("xp", "hp", "h3p", "qkp", "vtp", "atp", "pp"))
    wst, wst2, hidp, xsqp, scr = (pools[k] for k in
                                  ("wst", "wst2", "hidp", "xsqp", "scr"))
    vecp, rcpp, ddp = (pools[k] for k in ("vecp", "rcpp", "ddp"))
    ps_mm, ps_aux = pools["ps_mm"], pools["ps_aux"]
    dram = pools["dram"]

    ones = sb.tile([128, 2], FP32R, tag="ones")
    nc.sync.dma_start(out=ones, in_=io["ones_r"])
    ones2 = sb.tile([34, 128], FP32R, tag="ones2")
    nc.sync.dma_start(out=ones2, in_=io["ones2"])

    def load_w(src_ap, dt, bi, pool, eng=None):
        wt = pool.tile(list(src_ap.shape[1:]), dt, tag="w")
        (eng or nc.sync).dma_start(out=wt, in_=src_ap[bi])
        return wt

    # ---------------- Stage 0: adaLN (8-way col split + AllGather) ----------
    cT = sb.tile([128, 8, 4], BF16, tag="cT")
    nc.sync.dma_start(out=cT, in_=io["cT"])
    b_all = sb.tile([128, 62], FP32, tag="b_all")
    nc.sync.dma_start(out=b_all, in_=io["b_all"])
    b_qkT, b_projT, b_ctxT = b_all[:, 0:8], b_all[:, 8:16], b_all[:, 16:24]
    b_outT, b_fc1T, b_fc2T = b_all[:, 24:32], b_all[:, 32:48], b_all[:, 48:56]
    b_adaT = b_all[:, 56:62]
    cs = cT
    nc.scalar.activation(cs, cT, AF.Silu)

    # AllToAll: in row r = my col-slice of mod for batch r//2; out row j =
    # rank j's col-slice for MY batch. Layout core-independent -> SPMD-safe.
    ag_in = dram.tile([8, 768], FP32R, tag="ag_i")
    ag_out = dram.tile([8, 768], FP32R, tag="ag_o")
    ag_in_v = ag_in.rearrange("(b r) (t p) -> p t b r", p=128, b=4)
    magT = sb.tile([128, 6, 4], FP32R, tag="magT")
    for t in range(6):
        wt = load_w(io["w_ada"], BF16, t, wst)
        pm = ps_aux.tile([128, 4], FP32, tag="aux")
        for k in range(8):
            nc.tensor.matmul(pm, wt[:, k, :], cs[:, k, :], start=(k == 0), stop=(k == 7))
        nc.vector.tensor_scalar(magT[:, t, :], pm, b_adaT[:, t:t + 1], None,
                                op0=ALU.add)
    for t in range(6):
        for r in range(2):
            nc.scalar.dma_start(out=ag_in_v[:, t, :, r], in_=magT[:, t, :])
    if nocc:
        nc.gpsimd.dma_start(out=ag_out, in_=ag_in)
    else:
        nc.gpsimd.collective_compute(
            "AllToAll", ALU.bypass, replica_groups=ALL_GROUP,
            ins=[ag_in.opt()], outs=[ag_out.opt()])

    x_tiles = []
    for j in range(NCH):
        xt = xp.tile([128, T], FP32R, tag="x")
        nc.sync.dma_start(out=xt, in_=io["xT"][128 * j:128 * (j + 1), :])
        x_tiles.append(xt)

    # mod views: modT [128, 6, 8] feature-partition; kss rows (sc, sh, ones)
    # per LN group for the S-broadcast lhsT.
    modT = sb.tile([128, 8, 6], FP32R, tag="modT")
    nc.sync.dma_start(out=modT, in_=ag_out.rearrange("j (t p) -> p j t", p=128))
    ksf = ag_out.rearrange("j (t p) -> t j p", p=128)
    kss_t = sb.tile([66, 8, 128], FP32R, tag="kss")
    for i in range(3):
        nc.sync.dma_start(out=kss_t[32 * i:32 * i + 1, :, :],
                          in_=ksf[2 * i + 1:2 * i + 2])
        nc.sync.dma_start(out=kss_t[32 * i + 1:32 * i + 2, :, :],
                          in_=ksf[2 * i:2 * i + 1])
        nc.vector.tensor_scalar(kss_t[32 * i:32 * i + 1, :, :],
                                kss_t[32 * i:32 * i + 1, :, :], 1.0, None,
                                op0=ALU.add)
    kss = [kss_t[32 * i:32 * i + 2] for i in range(3)]
    ksc = sb.tile([128, 3, 8], FP32, tag="ksc")
    nc.vector.tensor_scalar(ksc, modT.rearrange("p j (g a) -> p g a j", a=2)[:, :, 1, :],
                            1.0, None, op0=ALU.add)
    eps_t = sb.tile([1, 1], FP32, tag="eps")
    nc.vector.memset(eps_t, EPS)
    expb_t = sb.tile([128, 1], FP32, tag="expb")
    nc.vector.memset(expb_t, EXPB)
    rhs33 = sb.tile([66, T], FP32R, tag="rhs33")
    for i in range(3):
        nc.sync.dma_start(out=rhs33[32 * i + 1:32 * i + 2, :],
                          in_=io["row_const"][1:2, :])

    # ---------------- LN + modulate ----------------
    # h = (x*ksc)*R - S ; R = bcast(rsig); S_j = (1+sc_j)*murs - sh_j
    # ones cols pre-scaled by 1/D so sum_ps/sq_ps hold mu and E[x^2] directly.
    def layer_norm_mod(x_tiles, grp, write_h):
        sum_ps = ps_mm.tile([1, T], FP32, tag="mm")
        sq_ps = ps_mm.tile([1, T], FP32, tag="mm")
        for j in range(NCH):
            xsq = xsqp.tile([128, T], FP32R, tag="xsq")
            nc.scalar.activation(xsq, x_tiles[j], AF.Square)
            for nb in range(2):
                s = slice(512 * nb, 512 * (nb + 1))
                nc.tensor.matmul(sum_ps[:, s], ones[:, 0:1], x_tiles[j][:, s],
                                 start=(j == 0), stop=(j == NCH - 1),
                                 skip_group_check=True)
                nc.tensor.matmul(sq_ps[:, s], ones[:, 1:2], xsq[:, s],
                                 start=(j == 0), stop=(j == NCH - 1),
                                 skip_group_check=True)
        musq = vecp.tile([1, T], FP32, tag="tmpv")
        nc.scalar.activation(musq, sum_ps, AF.Square)
        var = vecp.tile([1, T], FP32, tag="tmpv2")
        nc.vector.scalar_tensor_tensor(var, sq_ps, 1.0, musq,
                                       op0=ALU.mult, op1=ALU.subtract)
        sig = vecp.tile([1, T], FP32, tag="tmpv")
        nc.scalar.activation(sig, var, AF.Sqrt, bias=eps_t)
        rsig = vecp.tile([1, T], FP32R, tag="tmpv2")
        with nc.allow_low_precision(reason="fp32r rsig feeds broadcast matmul"):
            nc.vector.reciprocal(rsig, sig)
        r0 = 32 * grp
        nc.vector.tensor_tensor(rhs33[r0:r0 + 1, :], sum_ps, rsig, op=ALU.mult)
        Rh = []
        for nb in range(2):
            s = slice(512 * nb, 512 * (nb + 1))
            R = ps_aux.tile([128, 512], FP32, tag="aux", name=f"R{grp}{nb}")
            nc.tensor.matmul(R, ones2[0:1, :], rsig[:, s], start=True, stop=True)
            Rh.append(R)
        for j in range(NCH):
            for nb in range(2):
                s = slice(512 * nb, 512 * (nb + 1))
                S = ps_aux.tile([128, 512], FP32, tag="aux", name=f"S_{grp}_{j}_{nb}")
                nc.tensor.matmul(S, kss[grp][:, j, :],
                                 rhs33[32 * grp:32 * grp + 2, s],
                                 start=True, stop=True)
                t1 = scr.tile([128, 512], FP32, tag="t1")
                nc.vector.scalar_tensor_tensor(t1, x_tiles[j][:, s],
                                               ksc[:, grp, j:j + 1], Rh[nb],
                                               op0=ALU.mult, op1=ALU.mult)
                write_h(j, nb, t1, S)

    # ---------------- Stage 1: LN1 ----------------
    h1 = [hp.tile([128, 2, T], FP8, tag="h", name=f"h1_{g}") for g in range(4)]

    def _split_h(dst, nb, t1, S, j):
        if nb == 0 and j % 2 == 0:
            nc.vector.tensor_tensor(dst, t1, S, op=ALU.subtract)
        else:
            Sc = ddp.tile([128, 512], FP32, tag="dt")
            nc.scalar.copy(Sc, S)
            nc.gpsimd.tensor_tensor(dst, t1, Sc, op=ALU.subtract)

    def write_h1(j, nb, t1, S):
        s = slice(512 * nb, 512 * (nb + 1))
        _split_h(h1[j // 2][:, j % 2, s], nb, t1, S, j)

    layer_norm_mod(x_tiles, 0, write_h1)

    # ---------------- Stage 2: qkv ----------------
    bv_b = sb.tile([128, DL], FP32, tag="bv")
    bv_src = io["b_v"]
    nc.sync.dma_start(out=bv_b, in_=bass.AP(tensor=bv_src.tensor, offset=bv_src.offset,
                                            ap=[[0, 128]] + list(bv_src.ap[1:])))
    qkT = [None] * 8
    for mb in (0, 2, 1, 3):
        wt = load_w(io["w_qk"], FP8, mb, wst)
        for mm in range(2):
            m = 2 * mb + mm
            pm = ps_mm.tile([128, T], FP32, tag="mm")
            for nb in range(2):
                s = slice(512 * nb, 512 * (nb + 1))
                for g in range(4):
                    nc.tensor.matmul(pm[:, s], wt[:, g, :, 128 * mm:128 * (mm + 1)],
                                     h1[g][:, :, s], start=(g == 0), stop=(g == 3),
                                     perf_mode=DR)
            qk = qkp.tile([128, T], BF16, tag="qk")
            nc.vector.tensor_scalar(qk, pm, b_qkT[:, m:m + 1], None, op0=ALU.add)
            qkT[m] = qk
    # vT token-major, DR pairs [128, 2, HL, HD+1]
    wv_blk = [load_w(io["w_v"], FP8, b, wst) for b in range(2)]
    vdr = [vtp.tile([128, 2, HL, 72], FP8, tag="vt", name=f"vdr{i}")
           for i in range(4)]
    for m in range(NCH):
        pv = ps_aux.tile([128, DL], FP32, tag="aux")
        for b in range(2):
            for g in range(4):
                nc.tensor.matmul(pv[:, 256 * b:256 * (b + 1)],
                                 h1[g][:, :, 128 * m:128 * (m + 1)],
                                 wv_blk[b][:, g, :, :],
                                 start=(g == 0), stop=(g == 3), perf_mode=DR)
        nc.vector.tensor_tensor(vdr[m // 2][:, m % 2, :, 0:HD],
                                pv.rearrange("p (a b) -> p a b", a=HL),
                                bv_b.rearrange("p (a b) -> p a b", a=HL), op=ALU.add)
        nc.sync.dma_start(out=vdr[m // 2][:, m % 2, :, HD:HD + 1],
                          in_=io["ones_8"].rearrange("p (j o) -> p j o", o=1))

    # ---------------- cross-attn context (independent of x; fills stalls) ----
    teT = sb.tile([128, 3, 2, TLP], FP8, tag="teT")
    nc.sync.dma_start(out=teT, in_=io["teT"])
    ctx4 = [pools["ctxp"].tile([128, 2, TLP], FP8, tag="ctxT", name=f"ctx{g}")
            for g in range(4)]
    for mb in range(4):
        wt = load_w(io["w_ctx"], FP8, mb, wst)
        for mm in range(2):
            j = 2 * mb + mm
            pc = ps_aux.tile([128, TLP], FP32, tag="aux")
            for g in range(3):
                nc.tensor.matmul(pc, wt[:, g, :, 128 * mm:128 * (mm + 1)],
                                 teT[:, g, :, :], start=(g == 0), stop=(g == 2),
                                 perf_mode=DR)
            nc.vector.tensor_scalar(ctx4[j // 2][:, j % 2, :], pc,
                                    b_ctxT[:, j:j + 1], None, op0=ALU.add)
    # k_ctx feature-major [128, TLP] bf16 x4; v_ctx [80, HL, HD+1] bf16
    kcT = []
    for mb in range(2):
        wt = load_w(io["w_k"], FP8, mb, wst)
        for mm in range(2):
            m = 2 * mb + mm
            pk = ps_aux.tile([128, TLP], FP32, tag="aux")
            for g in range(4):
                nc.tensor.matmul(pk, wt[:, g, :, 128 * mm:128 * (mm + 1)],
                                 ctx4[g], start=(g == 0), stop=(g == 3), perf_mode=DR)
            kt = qkp.tile([128, TLP], BF16, tag="qkc")
            nc.scalar.copy(kt, pk)
            kcT.append(kt)
    wvc_blk = [load_w(io["w_vc"], FP8, b, wst) for b in range(2)]
    pvc = ps_aux.tile([TLP, DL], FP32, tag="aux")
    for b in range(2):
        for g in range(4):
            nc.tensor.matmul(pvc[:, 256 * b:256 * (b + 1)], ctx4[g],
                             wvc_blk[b][:, g, :, :], start=(g == 0), stop=(g == 3),
                             perf_mode=DR)
    vc = vtp.tile([128, HL, HD + 1], BF16, tag="vtc")
    nc.vector.memset(vc, 0.0)
    nc.vector.tensor_copy(vc[0:TL, :, 0:HD], pvc[0:TL].rearrange("p (a b) -> p a b", a=HL))
    nc.sync.dma_start(out=vc[0:TL, :, HD:HD + 1],
                      in_=io["ones_b"][0:TL, :].rearrange("p (j o) -> p j o", o=1))

    # ---------------- attention ----------------
    def attention_self(q_tiles, k_tiles):
        at = [atp.tile([128, 2, T], FP8, tag="at", name=f"at{i}") for i in range(2)]
        for h in range(HL):
            ti, off = h // 2, 64 * (h % 2)
            q_ap = q_tiles[ti][off:off + 64, :]
            pdr = [pp.tile([128, 2, T], FP8, tag="p", name=f"p{h}_{i}")
                   for i in range(4)]
            for m in range(8):
                ps_sc = ps_mm.tile([128, T], FP32, tag="mm")
                k_ap = k_tiles[ti][off:off + 64, 128 * m:128 * (m + 1)]
                for nb in range(2):
                    s = slice(512 * nb, 512 * (nb + 1))
                    nc.tensor.matmul(ps_sc[:, s], k_ap, q_ap[:, s],
                                     start=True, stop=True)
                nc.scalar.activation(pdr[m // 2][:, m % 2, :], ps_sc, AF.Exp,
                                     bias=expb_t, scale=float(HD) ** -0.5)
            for nb in range(2):
                s = slice(512 * nb, 512 * (nb + 1))
                po = ps_aux.tile([HD + 1, 512], FP32, tag="aux", name=f"po{h}{nb}")
                for mp in range(4):
                    nc.tensor.matmul(po, vdr[mp][:, :, h, 0:HD + 1],
                                     pdr[mp][:, :, s], start=(mp == 0),
                                     stop=(mp == 3), perf_mode=DR,
                                     skip_group_check=True)
                rcp = rcpp.tile([1, 512], FP32R, tag="rcp2", name=f"rcp{h}{nb}")
                with nc.allow_low_precision(reason="fp32r rcp feeds broadcast"):
                    nc.vector.reciprocal(rcp, po[HD:HD + 1, :])
                pb = ps_aux.tile([64, 512], FP32, tag="aux", name=f"pb{h}{nb}")
                nc.tensor.matmul(pb, ones2[0:1, 0:64], rcp, start=True, stop=True)
                rc = scr.tile([64, 512], FP32R, tag="t1")
                nc.vector.tensor_copy(rc, pb)
                nc.vector.tensor_tensor(at[ti // 2][off:off + 64, ti % 2, s],
                                        po[0:HD, :], rc, op=ALU.mult)
        return at

    def attention_cross(q_tiles, k_tiles):
        at = [atp.tile([128, 2, T], FP8, tag="at", name=f"atc{i}") for i in range(2)]
        for h in range(HL):
            ti, off = h // 2, 64 * (h % 2)
            ps_sc = ps_mm.tile([128, T], FP32, tag="mm")
            k_ap = k_tiles[ti][off:off + 64, :]
            for nb in range(2):
                s = slice(512 * nb, 512 * (nb + 1))
                nc.tensor.matmul(ps_sc[:TLP, s], k_ap, q_tiles[ti][off:off + 64, s],
                                 start=True, stop=True)
            pt = pp.tile([128, T], BF16, tag="p", name=f"pc{h}")
            nc.scalar.activation(pt[:TLP, :], ps_sc[:TLP, :], AF.Exp,
                                 bias=expb_t[:TLP], scale=float(HD) ** -0.5)
            for nb in range(2):
                s = slice(512 * nb, 512 * (nb + 1))
                po = ps_aux.tile([HD + 1, 512], FP32, tag="aux", name=f"poc{h}{nb}")
                nc.tensor.matmul(po, vc[0:TLP, h, :], pt[0:TLP, s],
                                 start=True, stop=True)
                rcp = rcpp.tile([1, 512], FP32R, tag="rcp2", name=f"rcpc{h}{nb}")
                with nc.allow_low_precision(reason="fp32r rcp feeds broadcast"):
                    nc.vector.reciprocal(rcp, po[HD:HD + 1, :])
                pb = ps_aux.tile([64, 512], FP32, tag="aux", name=f"pbc{h}{nb}")
                nc.tensor.matmul(pb, ones2[0:1, 0:64], rcp, start=True, stop=True)
                rc = scr.tile([64, 512], FP32R, tag="t1")
                nc.vector.tensor_copy(rc, pb)
                nc.vector.tensor_tensor(at[ti // 2][off:off + 64, ti % 2, s],
                                        po[0:HD, :], rc, op=ALU.mult)
        return at

    # row-parallel matmul + bf16 delta AllReduce + local residual add
    def row_parallel_reduce(w_name, bT_name, at, ar_tag):
        bT = b_projT if bT_name == 'b_projT' else b_outT
        ar_in = dram.tile([D, T], BF16, tag=ar_tag + "_i")
        ar_out = dram.tile([D, T], BF16, tag=ar_tag + "_o")
        for mb in range(2):
            wt = load_w(io[w_name], FP8, mb, wst)
            for mm in range(4):
                m = 4 * mb + mm
                pm = ps_mm.tile([128, T], FP32, tag="mm")
                for nb in range(2):
                    s = slice(512 * nb, 512 * (nb + 1))
                    for g in range(2):
                        nc.tensor.matmul(pm[:, s], wt[:, g, :, 128 * mm:128 * (mm + 1)],
                                         at[g][:, :, s], start=(g == 0), stop=(g == 1),
                                         perf_mode=DR)
                dd = scr.tile([128, T], BF16, tag="t1")
                nc.scalar.activation(dd, pm, AF.Identity, bias=bT[:, m:m + 1])
                nc.sync.dma_start(out=ar_in[128 * m:128 * (m + 1), :], in_=dd)
        for ch in range(4):
            sl = slice(256 * ch, 256 * (ch + 1))
            if nocc:
                nc.sync.dma_start(out=ar_out[sl, :], in_=ar_in[sl, :])
            else:
                nc.gpsimd.collective_compute(
                    "AllReduce", ALU.add, replica_groups=PAIR_GROUPS,
                    ins=[ar_in[sl, :].opt()], outs=[ar_out[sl, :].opt()])
        for m in range(NCH):
            dt = ddp.tile([128, T], BF16, tag="dt")
            nc.sync.dma_start(out=dt, in_=ar_out[128 * m:128 * (m + 1), :])
            eng = nc.vector if m % 2 == 0 else nc.gpsimd
            eng.tensor_tensor(x_tiles[m], x_tiles[m], dt, op=ALU.add)

    at1 = attention_self(qkT[0:4], qkT[4:8])
    row_parallel_reduce("w_proj", 'b_projT', at1, "arp")

    # ---------------- Stage 4: cross-attention ----------------
    h2 = [hp.tile([128, 2, T], FP8, tag="h", name=f"h2_{g}") for g in range(4)]

    def write_h2(j, nb, t1, S):
        s = slice(512 * nb, 512 * (nb + 1))
        _split_h(h2[j // 2][:, j % 2, s], nb, t1, S, j)

    layer_norm_mod(x_tiles, 1, write_h2)

    q2T = []
    for mb in range(2):
        wt = load_w(io["w_q"], FP8, mb, wst)
        for mm in range(2):
            m = 2 * mb + mm
            pm = ps_mm.tile([128, T], FP32, tag="mm")
            for nb in range(2):
                s = slice(512 * nb, 512 * (nb + 1))
                for g in range(4):
                    nc.tensor.matmul(pm[:, s], wt[:, g, :, 128 * mm:128 * (mm + 1)],
                                     h2[g][:, :, s], start=(g == 0), stop=(g == 3),
                                     perf_mode=DR)
            qt = qkp.tile([128, T], BF16, tag="qk")
            nc.scalar.copy(qt, pm)
            q2T.append(qt)

    at2 = attention_cross(q2T, kcT)
    row_parallel_reduce("w_out", 'b_outT', at2, "aro")

    # ---------------- Stage 5: MLP (bf16) ----------------
    mask = sb.tile([128, 1], FP32, tag="mask")
    nc.sync.dma_start(out=mask, in_=io["maskT"])

    h3 = [h3p.tile([128, T], BF16, tag="h3", name=f"h3_{j}") for j in range(NCH)]

    def write_h3(j, nb, t1, S):
        s = slice(512 * nb, 512 * (nb + 1))
        _split_h(h3[j][:, s], nb, t1, S, j)

    layer_norm_mod(x_tiles, 2, write_h3)

    for tb in range(2):
        s = slice(512 * tb, 512 * (tb + 1))
        hid8 = [hidp.tile([128, 2, 512], FP8, tag="hid", name=f"h8_{tb}_{i}")
                for i in range(4)]
        hidb = []
        for mb in range(8):
            wt = load_w(io["w_fc1"], BF16, mb, wst2)
            for mm in range(2):
                m = 2 * mb + mm
                pm = ps_aux.tile([128, 512], FP32, tag="aux")
                for k in range(NCH):
                    nc.tensor.matmul(pm, wt[:, k, 128 * mm:128 * (mm + 1)], h3[k][:, s],
                                     start=(k == 0), stop=(k == NCH - 1))
                if m < 8:
                    nc.scalar.activation(hid8[m // 2][:, m % 2, :], pm, AF.Gelu,
                                         bias=b_fc1T[:, m:m + 1])
                else:
                    ht = hidp.tile([128, 512], BF16, tag="hid")
                    nc.scalar.activation(ht, pm, AF.Gelu, bias=b_fc1T[:, m:m + 1])
                    hidb.append(ht)
        for mb in range(4):
            wta = load_w(io["w_fc2a"], FP8, mb, wst)
            wtb = load_w(io["w_fc2b"], BF16, mb, wst2)
            for mm in range(2):
                m = 2 * mb + mm
                pm = ps_mm.tile([128, 512], FP32, tag="mm")
                for g in range(4):
                    nc.tensor.matmul(pm, wta[:, g, :, 128 * mm:128 * (mm + 1)],
                                     hid8[g], start=(g == 0), stop=False,
                                     perf_mode=DR)
                for k in range(8):
                    nc.tensor.matmul(pm, wtb[:, k, 128 * mm:128 * (mm + 1)], hidb[k],
                                     start=False, stop=(k == 7))
                ob = scr.tile([128, 512], FP32, tag="t1")
                nc.scalar.activation(ob, pm, AF.Identity, bias=b_fc2T[:, m:m + 1])
                ot = scr.tile([128, 512], FP16, tag="t1")
                nc.vector.scalar_tensor_tensor(ot, x_tiles[m][:, s], mask, ob,
                                               op0=ALU.mult, op1=ALU.add)
                nc.sync.dma_start(out=io["out_xT"][128 * m:128 * (m + 1), s], in_=ot)


def build(nocc=False):
    nc = bacc.Bacc("TRN2", target_bir_lowering=False, debug=False,
                   num_devices=1 if nocc else 8)
    io = _declare(nc)
    with tile.TileContext(nc) as tc:
        import contextlib
        with contextlib.ExitStack() as ctx:
            def pool(name, bufs, space="SBUF"):
                return ctx.enter_context(tc.tile_pool(name=name, bufs=bufs, space=space))
            pools = {
                "sb": pool("sb", 1),
                "xp": pool("xp", 8),
                "hp": pool("hp", 4),
                "h3p": pool("h3p", 8),
                "qkp": pool("qkp", 8),
                "vtp": pool("vtp", 6),
                "atp": pool("atp", 3),
                "pp": pool("pp", 8),
                "wst": pool("wst", 4),
                "wst2": pool("wst2", 3),
                "hidp": pool("hidp", 12),
                "xsqp": pool("xsqp", 2),
                "scr": pool("scr", 5),
                "rcpp": pool("rcpp", 3),
                "ddp": pool("ddp", 6),
                "ctxp": pool("ctxp", 4),
                "vecp": pool("vecp", 2),
                "ps_mm": pool("ps_mm", 2, "PSUM"),
                "ps_aux": pool("ps_aux", 4, "PSUM"),
                "dram": pool("dram", 1, "DRAM"),
            }
            _emit(tc, io, pools, nocc=nocc)
    nc.compile()
    return nc


def pretile(w, mblk):
    """[K, M] -> [M//mblk, 128, K//128, mblk]"""
    K, M = w.shape
    v = w.reshape(K // 128, 128, M // mblk, mblk).transpose(2, 1, 0, 3)
    return np.ascontiguousarray(v)


def pretile_dr(w, mblk):
    """[K, M] -> [M//mblk, 128, K//256, 2, mblk] (DoubleRow k-pairs)"""
    K, M = w.shape
    v = w.reshape(K // 256, 2, 128, M // mblk, mblk).transpose(3, 2, 0, 1, 4)
    return np.ascontiguousarray(v)


def shard_inputs(inputs):
    f32 = np.float32
    bf16 = ml_dtypes.bfloat16
    f8 = ml_dtypes.float8_e4m3
    x = np.asarray(inputs["x"], f32)
    c = np.asarray(inputs["c"], f32)
    te = np.asarray(inputs["text_embed"], f32)
    W_ada, b_ada = np.asarray(inputs["W_ada"], f32), np.asarray(inputs["b_ada"], f32)
    W_qkv, b_qkv = np.asarray(inputs["W_qkv"], f32), np.asarray(inputs["b_qkv"], f32)
    W_proj, b_proj = np.asarray(inputs["W_proj"], f32), np.asarray(inputs["b_proj"], f32)
    W_ctx, b_ctx = np.asarray(inputs["W_ctx"], f32), np.asarray(inputs["b_ctx"], f32)
    W_q, W_k, W_v = (np.asarray(inputs[k], f32) for k in ("W_q", "W_k", "W_v"))
    W_out, b_out = np.asarray(inputs["W_out"], f32), np.asarray(inputs["b_out"], f32)
    W_fc1, b_fc1 = np.asarray(inputs["W_fc1"], f32), np.asarray(inputs["b_fc1"], f32)
    W_fc2, b_fc2 = np.asarray(inputs["W_fc2"], f32), np.asarray(inputs["b_fc2"], f32)

    cT = np.ascontiguousarray(c.T.reshape(8, 128, B).transpose(1, 0, 2))
    teTp = np.pad(te.transpose(0, 2, 1), ((0, 0), (0, 0), (0, TLP - TL)))

    maps = []
    for core in range(8):
        b, hf = core // 2, core % 2
        sl = slice(DL * hf, DL * (hf + 1))
        half = (lambda a: a) if hf == 0 else (lambda a: np.zeros_like(a))
        qs = slice(DL * hf, DL * (hf + 1))
        ks_ = slice(D + DL * hf, D + DL * (hf + 1))
        vs = slice(2 * D + DL * hf, 2 * D + DL * (hf + 1))
        # adaLN interleaved col slice: chunks cc with cc % 8 == core
        acols = np.concatenate([np.arange(128 * (8 * t + core), 128 * (8 * t + core) + 128)
                                for t in range(6)])
        m = {
            "xT": np.ascontiguousarray(x[b].T),
            "cT": cT.astype(bf16),
            "teT": np.ascontiguousarray(
                teTp[b].reshape(3, 2, 128, TLP).transpose(2, 0, 1, 3)).astype(f8),
            "w_ada": pretile(W_ada[:, acols].astype(bf16), 128),
            "b_all": np.ascontiguousarray(np.concatenate([
                np.concatenate([b_qkv[qs], b_qkv[ks_]]).reshape(8, 128).T,
                half(b_proj).reshape(8, 128).T,
                b_ctx.reshape(8, 128).T,
                half(b_out).reshape(8, 128).T,
                b_fc1[FFL * hf:FFL * (hf + 1)].reshape(16, 128).T,
                half(b_fc2).reshape(8, 128).T,
                b_ada[acols].reshape(6, 128).T], axis=1)).astype(f32),
            "w_qk": pretile_dr(np.concatenate(
                [W_qkv[:, qs], W_qkv[:, ks_]], axis=1).astype(f8), 256),
            "w_v": pretile_dr(W_qkv[:, vs].astype(f8), 256),
            "b_v": b_qkv[vs][None, :].copy(),
            "w_proj": pretile_dr(W_proj[sl, :].astype(f8), 512),
            "w_ctx": pretile_dr(W_ctx.astype(f8), 256),
            "w_q": pretile_dr(W_q[:, sl].astype(f8), 256),
            "w_k": pretile_dr(W_k[:, sl].astype(f8), 256),
            "w_vc": pretile_dr(W_v[:, sl].astype(f8), 256),
            "w_out": pretile_dr(W_out[sl, :].astype(f8), 512),
            "w_fc1": pretile(W_fc1[:, FFL * hf:FFL * (hf + 1)].astype(bf16), 256),
            "w_fc2a": pretile_dr(
                W_fc2[FFL * hf:FFL * hf + 1024, :].astype(f8), 256),
            "w_fc2b": pretile(
                W_fc2[FFL * hf + 1024:FFL * (hf + 1), :].astype(bf16), 256),
            "ones_r": np.full((128, 2), 1.0 / D, f32),
            "ones2": np.ascontiguousarray(np.concatenate(
                [np.ones((1, 128), f32), np.zeros((31, 128), f32),
                 np.repeat(np.eye(2, dtype=f32), 64, axis=1)])),
            "ones_8": np.ones((128, 8), f8),
            "ones_b": np.ones((128, 8), bf16),
            "maskT": np.full((128, 1), 1.0 - hf, f32),
            "row_const": np.concatenate([np.ones((1, 1024), f32),
                                         -np.ones((1, 1024), f32)]),
        }
        maps.append(m)
    return maps


_NC_CACHE = None


def kernel(**inputs):
    global _NC_CACHE
    if _NC_CACHE is None:
        _NC_CACHE = build()
    in_maps = shard_inputs(inputs)
    res = run_bass_kernel_spmd(_NC_CACHE, in_maps, core_ids=list(range(8)))
    out = np.empty((B, N, D), np.float32)
    for b in range(B):
        p0 = res.results[2 * b]["out_xT"]
        p1 = res.results[2 * b + 1]["out_xT"]
        out[b] = (p0.astype(np.float32) + p1.astype(np.float32)).T
    return out
